# revision 3
# baseline (speedup 1.0000x reference)
"""Block-DCT quantizer (8x8 DCT -> quant/dequant -> IDCT) on 8 Trainium2 cores.

Sharding: pure data parallel over batch. Core b processes x[b] = [3, 1024, 1024],
flattened to [3072, 1024], in 24 chunks of [128, 1024].

Per-chunk pipeline (all matmuls have the DCT matrix stationary and the data
streaming at N=512, so LDWEIGHTS amortizes):

    S1  colDCT      ps1 = Dbig @ X              [h,w] -> [h',w]
    E1  evac        y1  = bf16(ps1)                           (ACT)
    F1  32x32-block transpose (DVE): partition becomes (h' div 32, w mod 32).
        w mod 8 stays 8-aligned inside the partition dim, so the row DCT in
        this scrambled layout is the SAME block-diagonal matrix Dbig.
    S2  rowDCT/q    ps2 = (Dbig/qstep) @ y1t
    E2  quantize    q2  = bf16((ps2 + MAGIC) - MAGIC)  = rint  (DVE, exact)
    S3  rowIDCT*q   ps3 = (qstep*Dbig^T) @ q2
    E3  evac        z   = bf16(ps3)                           (ACT)
    F2  block transpose back (involution) -> [h', w]          (DVE)
    S4  colIDCT     ps4 = Dbig^T @ zt -> [h,w]
    E4  evac        o   = fp32(ps4)                           (ACT)

Input is cast fp32->bf16 for free by a gpsimd DMA. Quantized coefficients land
on exact integers (all zero for sane inputs), so bf16 intermediates cannot
perturb the rounding decision; the final IDCT output then matches fp32 refs.
"""
import math
import sys

sys.path.insert(0, "/opt/trn_rl_repo")

import ml_dtypes
import numpy as np

import concourse.bass as bass  # noqa: F401
import concourse.mybir as mybir
import concourse.tile as tile
from concourse import bacc, bass_utils

P = 128
CW = 1024        # chunk width
NB = 512         # matmul free-dim per PSUM bank
N_CORES = 8

_BUILD_CACHE = {}

MAGIC = float(np.float32(1.5 * 2**23))


def _dct_matrix(n: int) -> np.ndarray:
    k = np.arange(n, dtype=np.float64)[:, None]
    j = np.arange(n, dtype=np.float64)[None, :]
    d = np.cos(math.pi / n * (j + 0.5) * k)
    scale = np.full((n, 1), math.sqrt(2.0 / n))
    scale[0, 0] = math.sqrt(1.0 / n)
    return d * scale


def _build(rows: int, width: int):
    key = (rows, width)
    if key in _BUILD_CACHE:
        return _BUILD_CACHE[key]

    assert rows % P == 0 and width % CW == 0
    n_strips = rows // P
    n_wchunks = width // CW
    f32 = mybir.dt.float32
    bf16 = mybir.dt.bfloat16

    nc = bacc.Bacc("TRN2", target_bir_lowering=False, debug=False,
                   num_devices=N_CORES)
    x = nc.dram_tensor("x", [rows, width], f32, kind="ExternalInput").ap()
    ms = [
        nc.dram_tensor(f"m{i}", [P, P], bf16, kind="ExternalInput").ap()
        for i in range(1, 5)
    ]
    y = nc.dram_tensor("y", [rows, width], f32, kind="ExternalOutput").ap()

    with tile.TileContext(nc) as tc:
        with tc.tile_pool(name="consts", bufs=1) as cpool, \
             tc.tile_pool(name="io", bufs=3) as iopool, \
             tc.tile_pool(name="mid", bufs=3) as midpool, \
             tc.tile_pool(name="psum", bufs=4, space="PSUM") as psum:
            mt = []
            for i, m in enumerate(ms):
                t = cpool.tile([P, P], bf16, tag=f"m{i}")
                nc.sync.dma_start(out=t, in_=m)
                mt.append(t)
            m1t, m2t, m3t, m4t = mt

            def mm_stage(dst_ps, lhs_const, src_tile):
                for h in range(CW // NB):
                    sl = slice(h * NB, (h + 1) * NB)
                    nc.tensor.matmul(dst_ps[:, sl], lhsT=lhs_const,
                                     rhs=src_tile[:, sl], start=True, stop=True)

            for s in range(n_strips):
                for c in range(n_wchunks):
                    r0 = s * P
                    c0 = c * CW
                    xb = iopool.tile([P, CW], bf16, tag="xb")
                    nc.gpsimd.dma_start(out=xb, in_=x[r0:r0 + P, c0:c0 + CW])

                    ps1 = psum.tile([P, CW], f32, tag="ps")
                    mm_stage(ps1, m1t, xb)
                    y1 = midpool.tile([P, CW], bf16, tag="y1")
                    nc.scalar.copy(y1, ps1)
                    y1t = midpool.tile([P, CW], bf16, tag="y1t")
                    nc.vector.transpose(out=y1t, in_=y1)

                    ps2 = psum.tile([P, CW], f32, tag="ps")
                    mm_stage(ps2, m2t, y1t)
                    q2 = midpool.tile([P, CW], bf16, tag="q2")
                    nc.vector.tensor_scalar(
                        out=q2, in0=ps2, scalar1=MAGIC, scalar2=MAGIC,
                        op0=mybir.AluOpType.add, op1=mybir.AluOpType.subtract)

                    ps3 = psum.tile([P, CW], f32, tag="ps")
                    mm_stage(ps3, m3t, q2)
                    z = midpool.tile([P, CW], bf16, tag="z")
                    nc.scalar.copy(z, ps3)
                    zt = midpool.tile([P, CW], bf16, tag="zt")
                    nc.vector.transpose(out=zt, in_=z)

                    ps4 = psum.tile([P, CW], f32, tag="ps")
                    mm_stage(ps4, m4t, zt)
                    o = iopool.tile([P, CW], f32, tag="o")
                    nc.scalar.copy(o, ps4)
                    nc.sync.dma_start(out=y[r0:r0 + P, c0:c0 + CW], in_=o)

    nc.compile()
    _BUILD_CACHE[key] = nc
    return nc


def kernel(x: np.ndarray, block_size, qp, _trace: bool = False,
           _results_out: list | None = None) -> np.ndarray:
    n = int(block_size)
    qp = int(qp)
    b, ch, h, w = x.shape
    assert P % n == 0, f"block size {n} must divide {P}"
    # the 32x32 block-transpose keeps w mod 32 in the partition dim; the row
    # DCT stays block-diagonal iff n divides 32
    assert 32 % n == 0, f"block size {n} must divide 32"
    assert h % n == 0 and w % n == 0, "padding path not implemented"
    assert b == N_CORES, f"expected batch {N_CORES}, got {b}"
    rows = ch * h
    assert rows % P == 0 and w % CW == 0

    qstep = float(np.float32(2.0 ** ((qp - 4.0) / 6.0)))
    d = _dct_matrix(n)
    dbig = np.kron(np.eye(P // n), d)
    m1 = dbig.T            # colDCT:      out = Dbig @ X
    m2 = dbig.T / qstep    # rowDCT/q     (same Dbig in scrambled layout)
    m3 = qstep * dbig      # rowIDCT*q
    m4 = dbig              # colIDCT
    consts = {
        f"m{i}": np.ascontiguousarray(m.astype(ml_dtypes.bfloat16))
        for i, m in enumerate((m1, m2, m3, m4), start=1)
    }

    nc = _build(rows, w)
    x_np = np.asarray(x, dtype=np.float32)
    in_maps = [
        {"x": np.ascontiguousarray(x_np[i].reshape(rows, w)), **consts}
        for i in range(N_CORES)
    ]
    res = bass_utils.run_bass_kernel_spmd(
        nc, in_maps, core_ids=list(range(N_CORES)), trace=_trace)
    if _results_out is not None:
        _results_out.append(res)
    out = np.stack([res.results[i]["y"].reshape(ch, h, w)
                    for i in range(N_CORES)])
    return out
